# revision 48
# baseline (speedup 1.0000x reference)
"""Trainium2 Bass kernel for nn_DeepQNet_62268435857941 (GAT + DeepQNet head).

Math: with state s[b,:] (N,), W_gat (1,H*E), the GAT collapses because
Wh[b,h,n,e] = s[b,n] * W_gat[h,e] is rank-1 per head:
  a_i = c_src[h]*s_i,  b_j = c_tgt[h]*s_j,  x_ij = a_i + b_j
  m_ij = maskf_ij * exp(LeakyReLU(x_ij))
       = maskf_ij * [ sigma_ij * p_i q_j + (1-sigma_ij) * r_i u_j ]
  with sigma_ij = [x_ij >= 0] (exact selection identity for
  max(e^x, e^{0.2x})), p = e^{a}, q = e^{b}, r = e^{0.2a}, u = e^{0.2b}.

Key trick: the host bakes the mask INTO the sigma operand:
  msk_s[j,i] = s_i + BIG*(maskf_ij - 1)   (BIG=128, bf16)
so per (head, j-block) the masked selection matrix is ONE 4x-mode DVE op
  G = [msk_s >= thr'_j],  thr'_j = clip(-c_tgt*s_j / c_src, +-64)
(masked entries sit at ~-122 and always fail). The sign of c_src is
handled with runtime alpha/beta coefficients using mask sums recovered
from the SAME msk_s stream:  Mv = msum/BIG + Sum(v) * (1 - s_i/BIG),
  sigma-sums' = alpha*Gv + beta*Mv  (beta=1 swaps to the complement).
All (G v)/(M v) sums are PE matvecs; row->column conversion uses PE
transposes (identity rhs) deferred so the in-order PE never stalls.

Sharding: data-parallel over batch, core c <-> b = c, zero collectives.
"""
import os
import sys

sys.path.insert(0, "/opt/trn_rl_repo")

import numpy as np
import ml_dtypes

import concourse.bass as bass
import concourse.tile as tile
from concourse import mybir
from concourse.bass_utils import run_bass_kernel_spmd

B, N, H, E = 8, 2048, 4, 64
NJB = N // 128   # j blocks (partition dim)
NIC = N // 512   # i chunks (psum free dim)
N_CORES = 8
BIG = 128.0
CLMP = 64.0

F32 = mybir.dt.float32
F32R = mybir.dt.float32r
BF16 = mybir.dt.bfloat16
ACT = mybir.ActivationFunctionType
ALU = mybir.AluOpType
AX = mybir.AxisListType

MDT = BF16

# which sigma ops go to the Pool engine (per head, by j-block)
POOL_SIG = {1: {6, 12}, 2: {6, 12}, 3: {6, 12}}


def _split_sync_waits(nc, max_waits=1):
    """walrus in this env rejects >1 sync-wait per instruction; hoist the
    excess onto same-engine NoOps inserted right before the instruction."""
    n = 0
    for fn in nc.m.functions:
        for blk in fn.blocks:
            insts = blk.instructions
            i = 0
            while i < len(insts):
                inst = insts[i]
                si = inst.sync_info
                waits = list(si.on_wait) if si is not None else []
                if len(waits) > max_waits:
                    keep = waits[-max_waits:]
                    rest = waits[:-max_waits]
                    chunks = [rest[j:j + max_waits] for j in range(0, len(rest), max_waits)]
                    si.on_wait = keep
                    for k, chunk in enumerate(chunks):
                        nop = mybir.InstNoOp(
                            name=nc.get_next_instruction_name(),
                            engine=inst.engine,
                            sync_info=mybir.SyncInfo(on_wait=chunk, on_update=[]),
                            bass_nofuse=True,
                        )
                        insts.insert(i + k, nop)
                        nc.register_instruction(nop, overwrite=True)
                    i += len(chunks)
                    n += 1
                i += 1
    return n


def build():
    nc = bass.Bass()

    state_b = nc.dram_tensor("state_b", [1, N], F32, kind="ExternalInput")
    mskst = nc.dram_tensor("mskst", [N, N], MDT, kind="ExternalInput")
    strucT = nc.dram_tensor("strucT", [E, N], F32, kind="ExternalInput")
    cst = nc.dram_tensor("cst", [1, 6 * H], F32, kind="ExternalInput")
    arow = nc.dram_tensor("arow", [1, 48], F32, kind="ExternalInput")
    wq4 = nc.dram_tensor("wq4", [H, E], MDT, kind="ExternalInput")
    wsT = nc.dram_tensor("wsT", [E, E], F32, kind="ExternalInput")
    lin1T = nc.dram_tensor("lin1T", [E, E], F32, kind="ExternalInput")
    lin2T = nc.dram_tensor("lin2T", [E, E], F32, kind="ExternalInput")
    lin1b = nc.dram_tensor("lin1b", [E, 1], F32, kind="ExternalInput")
    lin2b = nc.dram_tensor("lin2b", [E, 1], F32, kind="ExternalInput")
    l3a = nc.dram_tensor("l3a", [E, 1], F32, kind="ExternalInput")
    l3b = nc.dram_tensor("l3b", [E, 1], F32, kind="ExternalInput")
    l3bias = nc.dram_tensor("l3bias", [1, 1], F32, kind="ExternalInput")
    ident = nc.dram_tensor("ident", [48, 16], F32, kind="ExternalInput")
    sums_d = nc.dram_tensor("sums_d", [1, 16], F32, kind="Internal")
    out_d = nc.dram_tensor("out", [1, N], F32, kind="ExternalOutput")

    with tile.TileContext(nc) as tc:
        with (
            tc.tile_pool(name="persist", bufs=1) as pp,
            tc.tile_pool(name="mask", bufs=NJB) as maskp,
            tc.tile_pool(name="sg", bufs=8) as sgp,
            tc.tile_pool(name="gsb", bufs=1) as gsbp,
            tc.tile_pool(name="small", bufs=6) as smp,
            tc.tile_pool(name="rba", bufs=2) as rbap,
            tc.tile_pool(name="big", bufs=7, space="PSUM") as bigp,
            tc.tile_pool(name="mcol", bufs=1, space="PSUM") as mcolp,
            tc.tile_pool(name="dram1", bufs=1, space="DRAM") as drp1,
        ):
            # ---------- prep (small DMAs first, then msk_s tiles) ----------
            state_cols = pp.tile([128, NJB], F32, tag="state_cols")
            nc.sync.dma_start(
                state_cols[:], state_b.rearrange("a (n p) -> (a p) n", p=128)
            )
            cst_b = pp.tile([128, 6 * H], F32, tag="cst_b")
            nc.sync.dma_start(cst_b[:], cst[0:1, :].partition_broadcast(128))
            # masked-state tiles, resident (the sigma operand)
            msk = []
            for jb in range(NJB):
                mt = maskp.tile([128, N], MDT, tag="mask", name=f"msk_{jb}")
                nc.sync.dma_start(mt[:], mskst[128 * jb:128 * (jb + 1), :])
                msk.append(mt)
                if jb == 3:
                    arow_b = pp.tile([128, 48], F32, tag="arow_b")
                    nc.sync.dma_start(arow_b[:],
                                      arow[0:1, :].partition_broadcast(128))
                    wq4_s = pp.tile([H, E], MDT, tag="wq4_s")
                    nc.sync.dma_start(wq4_s[:], wq4[:])
                    ident_s = pp.tile([48, 16], F32, tag="ident_s")
                    nc.sync.dma_start(ident_s[:], ident[:])

            wswst = pp.tile([E, E], F32R, tag="wswst")
            nc.gpsimd.dma_start(wswst[:], wsT[:])
            stacked = pp.tile([E, N], F32R, tag="stacked")
            nc.gpsimd.dma_start(stacked[:], strucT[:])
            lin1T_s = pp.tile([E, E], F32, tag="lin1T_s")
            nc.sync.dma_start(lin1T_s[:], lin1T[:])
            lin2T_s = pp.tile([E, E], F32R, tag="lin2T_s")
            nc.gpsimd.dma_start(lin2T_s[:], lin2T[:])
            lin1b_s = pp.tile([E, 1], F32, tag="lin1b_s")
            nc.sync.dma_start(lin1b_s[:], lin1b[:])
            lin2b_s = pp.tile([E, 1], F32, tag="lin2b_s")
            nc.sync.dma_start(lin2b_s[:], lin2b[:])
            l3a_s = pp.tile([E, 1], F32, tag="l3a_s")
            nc.sync.dma_start(l3a_s[:], l3a[:])
            l3b_s = pp.tile([E, 1], F32R, tag="l3b_s")
            nc.gpsimd.dma_start(l3b_s[:], l3b[:])
            l3bias_s = pp.tile([1, 1], F32, tag="l3bias_s")
            nc.sync.dma_start(l3bias_s[:], l3bias[:])

            # cst_b columns per head h: [csrc, ctgt, csrc02, ctgt02, -, coefp]
            cv = cst_b.rearrange("p (h k) -> p h k", h=H)

            # thr'_hj = clip(coefp_h * s_j, +-CLMP)
            thr = pp.tile([128, H * NJB], F32, tag="thr")
            thr_v = thr.rearrange("p (h n) -> p h n", h=H)
            for h in range(H):
                ttmp = smp.tile([128, NJB], F32, tag="ttmp", name=f"ttmp{h}")
                nc.vector.tensor_scalar(ttmp[:], state_cols[:],
                                        cv[:, h:h + 1, 5], CLMP,
                                        ALU.mult, ALU.min)
                nc.vector.tensor_scalar_max(thr_v[:, h, :], ttmp[:], -CLMP)

            # per-head exp factors in column layout + matmul lhsT tiles
            p_cols = pp.tile([128, H * NJB], F32, tag="p_cols")
            p_v = p_cols.rearrange("p (h n) -> p h n", h=H)
            r_cols = pp.tile([128, H * NJB], F32, tag="r_cols")
            r_v = r_cols.rearrange("p (h n) -> p h n", h=H)
            lhsG = pp.tile([128, H * NJB * 4], MDT, tag="lhsG")
            lhsG_v = lhsG.rearrange("p (h n k) -> p h n k", h=H, k=4)
            # M-pass lhsT: all 16 (head, quantity) columns at one jb,
            # materialized contiguously (walrus rejects 3D weight APs)
            lhsM = pp.tile([128, NJB * 16], MDT, tag="lhsM")
            lhsM_v = lhsM.rearrange("p (n c) -> p n c", c=16)

            for h in range(H):
                qc = smp.tile([128, NJB], F32, tag="qc", name=f"qc_{h}")
                nc.scalar.activation(qc[:], state_cols[:], ACT.Exp,
                                     scale=cv[:, h:h + 1, 1])
                uc = smp.tile([128, NJB], F32, tag="uc", name=f"uc_{h}")
                nc.scalar.activation(uc[:], state_cols[:], ACT.Exp,
                                     scale=cv[:, h:h + 1, 3])
                nc.scalar.activation(p_v[:, h, :], state_cols[:], ACT.Exp,
                                     scale=cv[:, h:h + 1, 0])
                nc.scalar.activation(r_v[:, h, :], state_cols[:], ACT.Exp,
                                     scale=cv[:, h:h + 1, 2])
                nc.vector.tensor_copy(lhsG_v[:, h, :, 0], qc[:])
                nc.vector.tensor_mul(lhsG_v[:, h, :, 1], qc[:], state_cols[:])
                nc.vector.tensor_copy(lhsG_v[:, h, :, 2], uc[:])
                nc.vector.tensor_mul(lhsG_v[:, h, :, 3], uc[:], state_cols[:])
                for k in range(4):
                    nc.vector.tensor_copy(lhsM_v[:, :, 4 * h + k],
                                          lhsG_v[:, h, :, k])

            # w_col = 1 - bf16(s)/BIG (matches the bf16 rounding in msk_s)
            s_cols16 = pp.tile([128, NJB], MDT, tag="s_cols16")
            nc.vector.tensor_copy(s_cols16[:], state_cols[:])
            w_col = pp.tile([128, NJB], F32, tag="w_col")
            nc.vector.tensor_scalar(w_col[:], s_cols16[:], -1.0 / BIG, 1.0,
                                    ALU.mult, ALU.add)

            onesc = pp.tile([128, 1], MDT, tag="onesc")
            nc.vector.memset(onesc[:], 1.0)

            s_all4 = pp.tile([H, N], MDT, tag="s_all4")
            s_dram = drp1.tile([H, N], MDT, tag="s_dram")
            mcols = mcolp.tile([128, 512], F32, tag="mcols")
            mcols_v = mcols.rearrange("p (t k) -> p t k", k=16)
            sums16 = pp.tile([1, 16], F32, tag="sums16")
            sumsb_bc = pp.tile([128, 16], F32, tag="sumsb_bc")

            head_state = {}

            def emit_copies(h):
                ps, _ = head_state[h]
                gsb = gsbp.tile([48, N], F32, tag="gsb", name=f"gsb_{h}")
                for ic in range(NIC):
                    sl = slice(512 * ic, 512 * (ic + 1))
                    if ic in (1, 3):
                        nc.vector.tensor_copy(gsb[0:4, sl], ps[ic][0:4, :])
                    else:
                        nc.scalar.copy(gsb[0:4, sl], ps[ic][0:4, :])
                    if h == 0:
                        if ic in (0, 1):
                            nc.vector.tensor_copy(gsb[32:48, sl],
                                                  ps[ic][32:48, :])
                        else:
                            nc.scalar.copy(gsb[32:48, sl], ps[ic][32:48, :])
                head_state[h] = (ps, gsb)

            def emit_msums(h0gsb):
                for t in range(NJB):
                    nc.tensor.matmul(
                        mcols_v[:, t, :], h0gsb[32:48, 128 * t:128 * (t + 1)],
                        ident_s[32:48, 0:16], is_transpose=True,
                        start=True, stop=True,
                    )

            def emit_assembly(h, ics=None, cols_tile=None, group_ics=True):
                _, gsb = head_state[h]
                if cols_tile is None:
                    cols_tile = bigp.tile(
                        [128, 512], F32, tag="big",
                        name=f"cols_{h}_{0 if ics is None else ics[0]}")
                cols = cols_tile
                cols_v = cols.rearrange("p (t k) -> p t k", k=4)
                ic_list = list(range(NIC)) if ics is None else ics
                for ic in ic_list:
                    for t in range(4 * ic, 4 * ic + 4):
                        nc.tensor.matmul(
                            cols_v[:, t, :], gsb[0:4, 128 * t:128 * (t + 1)],
                            ident_s[0:4, 0:4], is_transpose=True,
                            start=True, stop=True,
                        )
                for icg in ([ic_list] if group_ics else [[i] for i in ic_list]):
                    ic0 = icg[0]
                    nb = 4 * len(icg)
                    ts_ = slice(4 * ic0, 4 * ic0 + nb)
                    sfx = f"{h}_{ic0}"
                    # v1[k] = acoef*G[k] + bcoef*(w*Sum + msum/BIG)
                    v1 = []
                    for k in range(4):
                        c = 4 * h + k
                        wbs = smp.tile([128, nb], F32, tag="wbs",
                                       name=f"wb{sfx}_{k}")
                        nc.vector.tensor_scalar_mul(
                            wbs[:], w_col[:, ts_], sumsb_bc[:, c:c + 1])
                        mv = smp.tile([128, nb], F32, tag="mv",
                                      name=f"mv{sfx}_{k}")
                        nc.vector.scalar_tensor_tensor(
                            mv[:], mcols_v[:, ts_, c],
                            arow_b[:, 16 + c:17 + c], wbs[:],
                            ALU.mult, ALU.add)
                        vv = smp.tile([128, nb], F32, tag="vv",
                                      name=f"vv{sfx}_{k}")
                        nc.vector.scalar_tensor_tensor(
                            vv[:], cols_v[:, ts_, k],
                            arow_b[:, 32 + c:33 + c], mv[:],
                            ALU.mult, ALU.add)
                        v1.append(vv)
                    ta = smp.tile([128, nb], F32, tag="ta", name=f"ta{sfx}")
                    nc.vector.tensor_mul(ta[:], v1[0][:], p_v[:, h, ts_])
                    tb = smp.tile([128, nb], F32, tag="tb", name=f"tb{sfx}")
                    nc.vector.tensor_mul(tb[:], v1[2][:], r_v[:, h, ts_])
                    dcol = smp.tile([128, nb], F32, tag="dcol", name=f"dc{sfx}")
                    nc.vector.tensor_add(dcol[:], ta[:], tb[:])
                    tcq = smp.tile([128, nb], F32, tag="tcq", name=f"tq{sfx}")
                    nc.vector.tensor_mul(tcq[:], v1[1][:], p_v[:, h, ts_])
                    td = smp.tile([128, nb], F32, tag="td", name=f"td{sfx}")
                    nc.vector.tensor_mul(td[:], v1[3][:], r_v[:, h, ts_])
                    scol = smp.tile([128, nb], F32, tag="scol", name=f"sc{sfx}")
                    nc.vector.tensor_add(scol[:], tcq[:], td[:])
                    dinv = smp.tile([128, nb], F32, tag="dinv", name=f"di{sfx}")
                    nc.vector.reciprocal(dinv[:], dcol[:])
                    sfin16 = smp.tile([128, nb], MDT, tag="sfin16",
                                      name=f"sf{sfx}")
                    nc.vector.tensor_mul(sfin16[:], scol[:], dinv[:])
                    sl = slice(512 * ic0, 512 * (ic0 + len(icg)))
                    nc.sync.dma_start(
                        s_dram[h:h + 1, sl].rearrange("a (n p) -> (a p) n",
                                                      p=128),
                        sfin16[:],
                    )
                    if h == 3:
                        nc.gpsimd.dma_start(s_all4[h:h + 1, sl],
                                            s_dram[h:h + 1, sl])
                    else:
                        nc.sync.dma_start(s_all4[h:h + 1, sl],
                                          s_dram[h:h + 1, sl])

            # ---------- attention heads ----------
            xpre0_sb = pp.tile([E, N], F32R, tag="xpre0_sb")
            for h in range(H):
                ps = [bigp.tile([48, 512], F32, tag="big", name=f"ps_{h}_{ic}")
                      for ic in range(NIC)]
                head_state[h] = (ps, None)
                for jb in range(NJB):
                    if h == 0 and jb == 6:
                        # column sums of lhsG (for the M recovery)
                        sums_ps = bigp.tile([64, 512], F32, tag="big",
                                            name="sums_ps")
                        nc.tensor.matmul(sums_ps[0:1, 0:256], onesc[:],
                                         lhsG[:, :], start=True, stop=True)
                        sumtmp = pp.tile([1, 256], F32, tag="sumtmp")
                        nc.scalar.copy(sumtmp[:], sums_ps[0:1, 0:256])
                        for hh in range(H):
                            nc.vector.tensor_reduce(
                                sums16[0:1, 4 * hh:4 * hh + 4],
                                sumtmp[0:1, 64 * hh:64 * hh + 64].rearrange(
                                    "a (n k) -> a k n", k=4),
                                AX.X, ALU.add)
                        nc.sync.dma_start(sums_d[:], sums16[:])
                        sums_bc = pp.tile([128, 16], F32, tag="sums_bc")
                        nc.sync.dma_start(
                            sums_bc[:], sums_d[0:1, :].partition_broadcast(128))
                        nc.vector.tensor_mul(sumsb_bc[:], sums_bc[:],
                                             arow_b[:, 0:16])
                    if h == 1 and jb == 3:
                        emit_msums(head_state[0][1])
                        emit_assembly(0)
                    if h == 1 and jb in (6, 8, 10):
                        ic = (jb - 6) // 2
                        sl = slice(512 * ic, 512 * (ic + 1))
                        xp0 = bigp.tile([64, 512], F32, tag="big",
                                        name=f"xp0_{ic}")
                        nc.tensor.matmul(xp0[:], wswst[:], stacked[:, sl],
                                         start=True, stop=True)
                        nc.scalar.copy(xpre0_sb[:, sl], xp0[:])
                    if h == 2 and jb == 3:
                        emit_assembly(1)
                    if h == 3 and jb == 3:
                        emit_assembly(2)
                    sg = sgp.tile([128, N], MDT, tag="sg")
                    if jb in POOL_SIG.get(h, ()):
                        nc.gpsimd.tensor_scalar(
                            sg[:], msk[jb][:], thr_v[:, h, jb:jb + 1], None,
                            ALU.is_ge, ALU.bypass)
                    else:
                        nc.vector.tensor_scalar(
                            sg[:], msk[jb][:], thr_v[:, h, jb:jb + 1], None,
                            ALU.is_ge, ALU.bypass)
                    for ic in range(NIC):
                        nc.tensor.matmul(
                            ps[ic][0:4, :],
                            lhsG_v[:, h, jb, :],
                            sg[:, 512 * ic:512 * (ic + 1)],
                            start=(jb == 0), stop=(jb == NJB - 1),
                        )
                    if h == 0:
                        for ic in range(NIC):
                            nc.tensor.matmul(
                                ps[ic][32:48, :],
                                lhsM_v[:, jb, :],
                                msk[jb][:, 512 * ic:512 * (ic + 1)],
                                start=(jb == 0), stop=(jb == NJB - 1),
                            )
                emit_copies(h)
                if h == 1:
                    # last Ws@strucT chunk after h1's psum frees
                    sl = slice(512 * 3, 512 * 4)
                    xp0 = bigp.tile([64, 512], F32, tag="big", name="xp0_3")
                    nc.tensor.matmul(xp0[:], wswst[:], stacked[:, sl],
                                     start=True, stop=True)
                    nc.scalar.copy(xpre0_sb[:, sl], xp0[:])

            # ---------- tail ----------
            xT = xpre0_sb
            se_parts = pp.tile([E, NIC], F32, tag="se_parts")
            out_sb = pp.tile([1, N], F32, tag="out_sb")
            term = pp.tile([1, 1], F32, tag="term")

            # h3 assembly per i-chunk, fused with x = relu(xpre0 + WQ@s_all)
            cols3 = bigp.tile([128, 512], F32, tag="big", name="cols_3")
            for half in range(2):
                emit_assembly(3, ics=[2 * half, 2 * half + 1],
                              cols_tile=cols3)
                for ic in (2 * half, 2 * half + 1):
                    sl = slice(512 * ic, 512 * (ic + 1))
                    wqps = bigp.tile([64, 512], F32, tag="big",
                                     name=f"wqps_{ic}")
                    nc.tensor.matmul(wqps[:], wq4_s[:], s_all4[:, sl],
                                     start=True, stop=True)
                    xadd = rbap.tile([E, 512], F32, tag="xadd",
                                     name=f"xadd_{ic}")
                    nc.vector.scalar_tensor_tensor(
                        xadd[:], wqps[:], 0.0, xpre0_sb[:, sl],
                        ALU.add, ALU.add)
                    if ic % 2 == 0:
                        nc.scalar.activation(xT[:, sl], xadd[:], ACT.Relu,
                                             accum_out=se_parts[:, ic:ic + 1])
                    else:
                        nc.vector.tensor_scalar(
                            xT[:, sl], xadd[:], 0.0, 0.0, ALU.max, ALU.add,
                            accum_out=se_parts[:, ic:ic + 1])

            # beta_state scalar term
            s_emb = pp.tile([E, 1], F32, tag="s_emb")
            nc.vector.tensor_reduce(s_emb[:], se_parts[:], AX.X, ALU.add)
            ps_bs = bigp.tile([64, 512], F32, tag="big", name="ps_bs")
            nc.tensor.matmul(ps_bs[:, 0:1], lin1T_s[:], s_emb[:])
            rbs = pp.tile([E, 1], F32, tag="rbs")
            nc.vector.tensor_scalar(rbs[:], ps_bs[:, 0:1], lin1b_s[:], 0.0,
                                    ALU.add, ALU.max)
            ps_t1 = bigp.tile([64, 512], F32, tag="big", name="ps_t1")
            nc.tensor.matmul(ps_t1[0:1, 0:1], rbs[:], l3a_s[:])
            nc.vector.tensor_add(term[:], ps_t1[0:1, 0:1], l3bias_s[:])

            # beta_action chain per chunk; +term via Act bias / DVE alternating
            for ic in range(NIC):
                sl = slice(512 * ic, 512 * (ic + 1))
                ps_ba = bigp.tile([64, 512], F32, tag="big", name=f"ps_ba_{ic}")
                nc.tensor.matmul(ps_ba[:], lin2T_s[:], xT[:, sl])
                rba = rbap.tile([E, 512], F32R, tag="rba")
                if ic % 2 == 0:
                    nc.scalar.activation(rba[:], ps_ba[:], ACT.Relu,
                                         bias=lin2b_s[:])
                else:
                    nc.vector.tensor_scalar(rba[:], ps_ba[:], lin2b_s[:], 0.0,
                                            ALU.add, ALU.max)
                ps_c = bigp.tile([64, 512], F32, tag="big", name=f"ps_c_{ic}")
                nc.tensor.matmul(ps_c[0:1, :], l3b_s[:], rba[:])
                if ic % 2 == 0:
                    nc.scalar.activation(out_sb[:, sl], ps_c[0:1, :],
                                         ACT.Identity, bias=term[:])
                else:
                    nc.vector.tensor_scalar_add(out_sb[:, sl], ps_c[0:1, :],
                                                term[:])
            nc.sync.dma_start(out_d[0:1, 0:1024], out_sb[0:1, 0:1024])
            nc.sync.dma_start(out_d[0:1, 1024:2048], out_sb[0:1, 1024:2048])

    _split_sync_waits(nc)
    return nc


_nc_cache = None


def _get_nc():
    global _nc_cache
    if _nc_cache is None:
        _nc_cache = build()
    return _nc_cache


def make_in_maps(state, strucEmb, adj_mask, W_gat, att, Ws, Wst,
                 lin1_w, lin1_b, lin2_w, lin2_b, lin3_w, lin3_b):
    state = np.asarray(state, np.float32)
    adj_mask = np.asarray(adj_mask)
    mdt_np = ml_dtypes.bfloat16 if MDT == BF16 else np.float32
    # maskB[j,i] = BIG*(maskf[i,j] - 1)  in {-BIG, 0}
    maskB = (np.float32(BIG) * ((~adj_mask).T.astype(np.float32) - 1.0))
    ident = np.zeros((48, 16), np.float32)
    ident[0:4, 0:4] = np.eye(4)
    ident[32:48, 0:16] = np.eye(16)

    wg = np.asarray(W_gat, np.float64).reshape(H, E)
    attn = np.asarray(att, np.float64)
    csrc = (wg * attn[:, :E, 0]).sum(1)
    ctgt = (wg * attn[:, E:, 0]).sum(1)
    csg = np.where(csrc >= 0, np.maximum(csrc, 1e-9), np.minimum(csrc, -1e-9))
    coefp = -ctgt / csg
    cstv = np.stack([csrc, ctgt, 0.2 * csrc, 0.2 * ctgt,
                     np.sign(csg), coefp], axis=1)
    alpha = np.where(csrc >= 0, 1.0, -1.0)
    beta = (1.0 - alpha) / 2.0
    gamma = 1.0 - beta
    # arow: [0:16] bcoef, [16:32] bcoef/BIG, [32:48] acoef, order c = 4h+k
    bcoef = np.stack([beta, beta, gamma, gamma], axis=1).reshape(-1)
    acoef = np.stack([alpha, alpha, -alpha, -alpha], axis=1).reshape(-1)
    arowv = np.concatenate([bcoef, bcoef / BIG, acoef]).astype(np.float32)

    wq4v = (wg / H) @ np.asarray(Wst, np.float64).T

    common = dict(
        strucT=np.ascontiguousarray(np.asarray(strucEmb, np.float32).T),
        cst=cstv.astype(np.float32).reshape(1, 6 * H),
        arow=arowv.reshape(1, 48),
        wq4=np.ascontiguousarray(wq4v.astype(mdt_np)),
        wsT=np.ascontiguousarray(np.asarray(Ws, np.float32).T),
        lin1T=np.ascontiguousarray(np.asarray(lin1_w, np.float32).T),
        lin2T=np.ascontiguousarray(np.asarray(lin2_w, np.float32).T),
        lin1b=np.asarray(lin1_b, np.float32).reshape(E, 1),
        lin2b=np.asarray(lin2_b, np.float32).reshape(E, 1),
        l3a=np.ascontiguousarray(np.asarray(lin3_w, np.float32)[0, :E].reshape(E, 1)),
        l3b=np.ascontiguousarray(np.asarray(lin3_w, np.float32)[0, E:].reshape(E, 1)),
        l3bias=np.asarray(lin3_b, np.float32).reshape(1, 1),
        ident=ident,
    )
    in_maps = []
    for c in range(N_CORES):
        srow = state[c].astype(mdt_np).astype(np.float32)
        mskstv = (srow[None, :] + maskB).astype(mdt_np)
        in_maps.append(dict(common, state_b=state[c:c + 1], mskst=mskstv))
    return in_maps


def kernel(**inputs):
    nc = _get_nc()
    in_maps = make_in_maps(**inputs)
    res = run_bass_kernel_spmd(nc, in_maps, list(range(N_CORES)))
    kernel._last_results = res
    out = np.stack([res.results[c]["out"].reshape(N, 1) for c in range(N_CORES)])
    return out.astype(np.float32)
